# revision 27
# baseline (speedup 1.0000x reference)
"""Trainium2 Bass kernel for DigitConvolutionalModel.

Math: logits = relu(conv2d_valid(x.reshape(B,28,28), conv_w).reshape(B,676) @ W1 + b1) @ W2 + b2

Optimizations:
  1. The valid 3x3 conv is linear in x, so it folds into W1 on host:
     feat @ W1 == x @ (C @ W1) where C[784,676] scatters conv_w taps.
     The device then runs two dense matmuls per batch shard:
       h = relu(x @ W1eff + b1);  logits = h @ W2 + b2
  2. Sharding layout: batch 32768 split as 8 x 4096 across cores; each
     shard is fed to its core pre-transposed and pre-tiled
     ([nblk, 112, 7, 512]) so every 512-column block is one fully
     contiguous 1.6MB DMA and the contraction dim lands on SBUF
     partitions with no on-device transposes of x.
  3. Matmuls run in float32r (PE fast-fp32, ~12-bit mantissa, one pass
     per row vs two for fp32). DRAM tensors are declared float32r so
     the data flows DMA->SBUF->PE with no rounding pass (the PE rounds
     internally; measured rel err ~2e-4).
  4. All small weights are packed into two host-prepared blobs loaded
     as single contiguous DMAs on the GPSIMD (SWDGE) queue so they
     never block the x stream on the HWDGE rings.

Device kernel (per core, per 512-column block):
  - one DMA pulls xT block [112 part, 7 k-chunks, 512 batch]
  - MM1: hT[256,512] = W1eff.T @ xT accumulated over 7 k-chunks
  - ACT: relu(hT + b1) PSUM->SBUF, output float32r
  - MM2: logitsT[10,512] over 2 k-chunks of W2
  - DVE: + b2 (per-partition scalar add) PSUM->SBUF
  - PE transpose-mode: logitsT -> [512, 10], DVE copy, DMA out
"""
import ml_dtypes
import numpy as np

import concourse.bacc as bacc
import concourse.mybir as mybir
from concourse.tile import TileContext
from concourse.bass_utils import run_bass_kernel_spmd

B = 32768
IMG = 28
KSZ = 3
OUT_HW = IMG - KSZ + 1  # 26
FEAT = OUT_HW * OUT_HW  # 676
PIX = IMG * IMG  # 784
HID = 256
NCLS = 10
N_CORES = 8
BC = B // N_CORES  # 4096 rows per core
NBLK_COLS = 512  # batch columns per pipeline block (1 PSUM bank of fp32)
KCH = 112  # 784 = 7 * 112 contraction chunks
NKC = PIX // KCH  # 7
# wb blob layout (per partition, fp32 words): b1[2] | b2
WB_B1 = 0          # [128, 2]
WB_B2 = 2          # [128, 1] (only partitions 0..9 used)
WB_W = 3

f32 = mybir.dt.float32
f32r = mybir.dt.float32r
bf16 = mybir.dt.bfloat16
AF = mybir.ActivationFunctionType

# dtype knobs: bf16 halves both DMA bytes and PE stream cycles vs f32r.
# measured end-to-end rel err ~3e-3 (vs ~2e-4 full-f32r).
X_DT = bf16
W_DT = bf16
H_DT = bf16

_CACHE = {}


def _build(bc=BC, reps=1, bench_internal_x=False):
    """Build the single-core Bass program (SPMD across 8 cores).

    reps > 1 unrolls the whole pipeline multiple times over the same
    input (benchmark-only; output identical since it is rewritten).
    bench_internal_x makes xT an internal DRAM tensor (garbage contents)
    so benchmark calls skip the 13MB/core upload; timing is unaffected.
    """
    nblk = bc // NBLK_COLS
    nc = bacc.Bacc()
    # xT is host-prearranged as [nblk, 112, 7, 512]: block-major, partition
    # k, k-chunk c, batch b -> each block is one fully contiguous 1.6MB DMA.
    xt_shape = [nblk, KCH, NKC, NBLK_COLS]
    if bench_internal_x:
        xT = nc.dram_tensor("xT_int", xt_shape, X_DT)[:]
    else:
        xT = nc.declare_dram_parameter("xT", xt_shape, X_DT, isOutput=False)
    # w1 blob: host-prearranged [112, 7, 256] (chunk-major per partition)
    w1e = nc.declare_dram_parameter("w1b", [KCH, NKC, HID], W_DT, isOutput=False)
    # w2 blob: [128, 2, 10] f32r (chunk-major per partition)
    w2 = nc.declare_dram_parameter("w2b", [128, 2, NCLS], W_DT, isOutput=False)
    # small-weights blob: [128, WB_W] fp32, see WB_* offsets
    wb = nc.declare_dram_parameter("wb", [128, WB_W], f32, isOutput=False)
    # output is logitsT [10, bc]; host transposes back
    out = nc.declare_dram_parameter("out", [NCLS, bc], f32, isOutput=True)

    with TileContext(nc) as tc:
        with (
            tc.tile_pool(name="weights", bufs=1) as wpool,
            tc.tile_pool(name="xt_sb", bufs=6) as xtpool,
            tc.tile_pool(name="h_sb", bufs=4) as hpool,
            tc.tile_pool(name="h_ps", bufs=3, space="PSUM") as hps,
            tc.tile_pool(name="log_ps", bufs=2, space="PSUM") as logps,
        ):
            # ---- one-time weight staging: 2 contiguous DMAs on SWDGE ----
            w1_sb = wpool.tile([KCH, NKC, HID], W_DT)
            nc.sync.dma_start(out=w1_sb[:], in_=w1e[:])
            w2_sb = wpool.tile([128, 2, NCLS], W_DT)
            nc.scalar.dma_start(out=w2_sb[:], in_=w2[:])
            wb_sb = wpool.tile([128, WB_W], f32)
            nc.scalar.dma_start(out=wb_sb[:], in_=wb[:])
            b1_sb = wb_sb[:, WB_B1:WB_B2]
            b2_sb = wb_sb[:NCLS, WB_B2:WB_W]
            # all blocks' logitsT accumulate here; one DMA at the end
            log_all = wpool.tile([NCLS, bc], f32)

            # ---- main pipeline over 512-column blocks ----
            # MM2 for block n is emitted during block n+1's MM1 so the PE
            # never waits on the relu round-trip.
            first_block = True
            pending = None  # (hs, b0) awaiting MM2

            def emit_mm2(hs, b0):
                log_ps = logps.tile([NCLS, NBLK_COLS], f32)
                for mc in range(2):
                    nc.tensor.matmul(
                        log_ps[:],
                        w2_sb[:, mc, :],
                        hs[mc][:],
                        start=(mc == 0),
                        stop=(mc == 1),
                    )
                nc.vector.tensor_scalar_add(
                    out=log_all[:, b0 : b0 + NBLK_COLS],
                    in0=log_ps[:],
                    scalar1=b2_sb[:, 0:1],
                )
                nc.gpsimd.dma_start(
                    out=out[:, b0 : b0 + NBLK_COLS],
                    in_=log_all[:, b0 : b0 + NBLK_COLS],
                )

            for blk in range(nblk * reps):
                blk = blk % nblk
                b0 = blk * NBLK_COLS

                xt = xtpool.tile([KCH, NKC, NBLK_COLS], X_DT, tag="xt")
                if first_block:
                    # split into per-chunk DMAs so MM1 kc=0 starts sooner
                    for kc in range(NKC):
                        eng = nc.sync if kc % 2 == 0 else nc.scalar
                        eng.dma_start(out=xt[:, kc, :], in_=xT[blk, :, kc, :])
                    first_block = False
                else:
                    eng = nc.sync if blk % 2 == 0 else nc.scalar
                    eng.dma_start(out=xt[:], in_=xT[blk])

                # MM1 + fused bias/relu -> h chunks [128, 512]
                hs = []
                for mc in range(2):
                    h_ps = hps.tile([128, NBLK_COLS], f32)
                    for kc in range(NKC):
                        nc.tensor.matmul(
                            h_ps[:],
                            w1_sb[:, kc, mc * 128 : (mc + 1) * 128],
                            xt[:, kc, :],
                            start=(kc == 0),
                            stop=(kc == NKC - 1),
                        )
                    h_sb = hpool.tile([128, NBLK_COLS], H_DT, tag="h")
                    nc.scalar.activation(
                        h_sb[:], h_ps[:], AF.Relu, bias=b1_sb[:, mc : mc + 1]
                    )
                    hs.append(h_sb)
                    if mc == 0 and pending is not None:
                        emit_mm2(*pending)
                        pending = None
                pending = (hs, b0)

            emit_mm2(*pending)

    nc.compile()
    return nc


def _fold_conv_into_w1(conv_w, W1):
    """W1eff[784, 256] such that x @ W1eff == conv(x) flattened @ W1."""
    conv_w = np.asarray(conv_w, dtype=np.float64)
    W1 = np.asarray(W1, dtype=np.float64)
    C = np.zeros((IMG, IMG, OUT_HW, OUT_HW), dtype=np.float64)
    oi = np.arange(OUT_HW)[:, None]
    oj = np.arange(OUT_HW)[None, :]
    for ki in range(KSZ):
        for kj in range(KSZ):
            C[oi + ki, oj + kj, oi, oj] = conv_w[ki, kj]
    W1eff = C.reshape(PIX, FEAT) @ W1
    return np.ascontiguousarray(W1eff, dtype=np.float32)


def _pack_weights(w1e, b1, W2, b2):
    # w1 blob [112, 7, 256]: chunk-major per partition (matches w1_sb)
    np_wdt = mybir.dt.np(W_DT)
    w1b = np.ascontiguousarray(
        w1e.reshape(NKC, KCH, HID).transpose(1, 0, 2).astype(np_wdt)
    )
    w2b = np.ascontiguousarray(
        W2.reshape(2, 128, NCLS).transpose(1, 0, 2).astype(np_wdt)
    )
    wb = np.zeros((128, WB_W), dtype=np.float32)
    wb[:, WB_B1:WB_B2] = b1.reshape(2, 128).T
    wb[:NCLS, WB_B2] = b2
    return w1b, w2b, wb


def kernel(x, conv_w, W1, b1, W2, b2, _bc=BC, _trace=False):
    x = np.asarray(x, dtype=np.float32)
    w1e = _fold_conv_into_w1(conv_w, W1)
    b1 = np.asarray(b1, dtype=np.float32)
    W2 = np.asarray(W2, dtype=np.float32)
    b2 = np.asarray(b2, dtype=np.float32)
    w1b, w2b, wb = _pack_weights(w1e, b1, W2, b2)

    n_cores = x.shape[0] // _bc
    if _bc not in _CACHE:
        _CACHE[_bc] = _build(_bc)
    nc = _CACHE[_bc]

    nblk = _bc // NBLK_COLS
    in_maps = [
        {
            # [nblk, 512, 7, 112] -> [nblk, 112, 7, 512]; see _build
            "xT": np.ascontiguousarray(
                x[c * _bc : (c + 1) * _bc]
                .reshape(nblk, NBLK_COLS, NKC, KCH)
                .transpose(0, 3, 2, 1)
                .astype(mybir.dt.np(X_DT))
            ),
            "w1b": w1b,
            "w2b": w2b,
            "wb": wb,
        }
        for c in range(n_cores)
    ]
    res = run_bass_kernel_spmd(
        nc, in_maps, core_ids=list(range(n_cores)), trace=_trace
    )
    # device layout logitsT [10, bc] -> [bc, 10]
    out = np.concatenate(
        [np.ascontiguousarray(res.results[c]["out"].T) for c in range(n_cores)],
        axis=0,
    )
    if _trace:
        return out, res
    return out


# revision 28
# speedup vs baseline: 1.0704x; 1.0704x over previous
"""Trainium2 Bass kernel for DigitConvolutionalModel.

Math: logits = relu(conv2d_valid(x.reshape(B,28,28), conv_w).reshape(B,676) @ W1 + b1) @ W2 + b2

Optimizations:
  1. The valid 3x3 conv is linear in x, so it folds into W1 on host:
     feat @ W1 == x @ (C @ W1) where C[784,676] scatters conv_w taps.
     The device then runs two dense matmuls per batch shard:
       h = relu(x @ W1eff + b1);  logits = h @ W2 + b2
  2. Sharding layout: batch 32768 split as 8 x 4096 across cores; each
     shard is fed to its core pre-transposed and pre-tiled
     ([nblk, 112, 7, 512]) so every 512-column block is one fully
     contiguous 1.6MB DMA and the contraction dim lands on SBUF
     partitions with no on-device transposes of x.
  3. Matmuls run in float32r (PE fast-fp32, ~12-bit mantissa, one pass
     per row vs two for fp32). DRAM tensors are declared float32r so
     the data flows DMA->SBUF->PE with no rounding pass (the PE rounds
     internally; measured rel err ~2e-4).
  4. All small weights are packed into two host-prepared blobs loaded
     as single contiguous DMAs on the GPSIMD (SWDGE) queue so they
     never block the x stream on the HWDGE rings.

Device kernel (per core, per 512-column block):
  - one DMA pulls xT block [112 part, 7 k-chunks, 512 batch]
  - MM1: hT[256,512] = W1eff.T @ xT accumulated over 7 k-chunks
  - ACT: relu(hT + b1) PSUM->SBUF, output float32r
  - MM2: logitsT[10,512] over 2 k-chunks of W2
  - DVE: + b2 (per-partition scalar add) PSUM->SBUF
  - PE transpose-mode: logitsT -> [512, 10], DVE copy, DMA out
"""
import ml_dtypes
import numpy as np

import concourse.bacc as bacc
import concourse.mybir as mybir
from concourse.tile import TileContext
from concourse.bass_utils import run_bass_kernel_spmd

B = 32768
IMG = 28
KSZ = 3
OUT_HW = IMG - KSZ + 1  # 26
FEAT = OUT_HW * OUT_HW  # 676
PIX = IMG * IMG  # 784
HID = 256
NCLS = 10
N_CORES = 8
BC = B // N_CORES  # 4096 rows per core
NBLK_COLS = 512  # batch columns per pipeline block (1 PSUM bank of fp32)
KCH = 112  # 784 = 7 * 112 contraction chunks
NKC = PIX // KCH  # 7
# wb blob layout (per partition, fp32 words): b1[2] | b2
WB_B1 = 0          # [128, 2]
WB_B2 = 2          # [128, 1] (only partitions 0..9 used)
WB_W = 3

f32 = mybir.dt.float32
f32r = mybir.dt.float32r
bf16 = mybir.dt.bfloat16
AF = mybir.ActivationFunctionType

# dtype knobs: bf16 halves both DMA bytes and PE stream cycles vs f32r.
# measured end-to-end rel err ~3e-3 (vs ~2e-4 full-f32r).
X_DT = bf16
W_DT = bf16
H_DT = bf16

_CACHE = {}


def _build(bc=BC, reps=1, bench_internal_x=False):
    """Build the single-core Bass program (SPMD across 8 cores).

    reps > 1 unrolls the whole pipeline multiple times over the same
    input (benchmark-only; output identical since it is rewritten).
    bench_internal_x makes xT an internal DRAM tensor (garbage contents)
    so benchmark calls skip the 13MB/core upload; timing is unaffected.
    """
    nblk = bc // NBLK_COLS
    nc = bacc.Bacc()
    # xT is host-prearranged as [nblk, 112, 7, 512]: block-major, partition
    # k, k-chunk c, batch b -> each block is one fully contiguous 1.6MB DMA.
    xt_shape = [nblk, KCH, NKC, NBLK_COLS]
    if bench_internal_x:
        xT = nc.dram_tensor("xT_int", xt_shape, X_DT)[:]
    else:
        xT = nc.declare_dram_parameter("xT", xt_shape, X_DT, isOutput=False)
    # w1 blob: host-prearranged [112, 7, 256] (chunk-major per partition)
    w1e = nc.declare_dram_parameter("w1b", [KCH, NKC, HID], W_DT, isOutput=False)
    # w2 blob: [128, 2, 10] f32r (chunk-major per partition)
    w2 = nc.declare_dram_parameter("w2b", [128, 2, NCLS], W_DT, isOutput=False)
    # small-weights blob: [128, WB_W] fp32, see WB_* offsets
    wb = nc.declare_dram_parameter("wb", [128, WB_W], f32, isOutput=False)
    # output is logitsT [10, bc]; host transposes back
    out = nc.declare_dram_parameter("out", [NCLS, bc], f32, isOutput=True)

    with TileContext(nc) as tc:
        with (
            tc.tile_pool(name="weights", bufs=1) as wpool,
            tc.tile_pool(name="xt_sb", bufs=6) as xtpool,
            tc.tile_pool(name="h_sb", bufs=4) as hpool,
            tc.tile_pool(name="h_ps", bufs=3, space="PSUM") as hps,
            tc.tile_pool(name="log_ps", bufs=2, space="PSUM") as logps,
        ):
            # ---- one-time weight staging: 2 contiguous DMAs on SWDGE ----
            w1_sb = wpool.tile([KCH, NKC, HID], W_DT)
            nc.sync.dma_start(out=w1_sb[:], in_=w1e[:])
            w2_sb = wpool.tile([128, 2, NCLS], W_DT)
            nc.scalar.dma_start(out=w2_sb[:], in_=w2[:])
            wb_sb = wpool.tile([128, WB_W], f32)
            nc.scalar.dma_start(out=wb_sb[:], in_=wb[:])
            b1_sb = wb_sb[:, WB_B1:WB_B2]
            b2_sb = wb_sb[:NCLS, WB_B2:WB_W]
            # all blocks' logitsT accumulate here; one DMA at the end
            log_all = wpool.tile([NCLS, bc], f32)

            # ---- main pipeline over 512-column blocks ----
            # MM2 for block n is emitted during block n+1's MM1 so the PE
            # never waits on the relu round-trip.
            first_block = True
            pending = None  # (hs, b0) awaiting MM2

            def emit_mm2(hs, b0):
                log_ps = logps.tile([NCLS, NBLK_COLS], f32)
                for mc in range(2):
                    nc.tensor.matmul(
                        log_ps[:],
                        w2_sb[:, mc, :],
                        hs[mc][:],
                        start=(mc == 0),
                        stop=(mc == 1),
                    )
                nc.vector.tensor_scalar_add(
                    out=log_all[:, b0 : b0 + NBLK_COLS],
                    in0=log_ps[:],
                    scalar1=b2_sb[:, 0:1],
                )
                nc.gpsimd.dma_start(
                    out=out[:, b0 : b0 + NBLK_COLS],
                    in_=log_all[:, b0 : b0 + NBLK_COLS],
                )

            for blk in range(nblk * reps):
                blk = blk % nblk
                b0 = blk * NBLK_COLS

                xt = xtpool.tile([KCH, NKC, NBLK_COLS], X_DT, tag="xt")
                if first_block:
                    # split into per-chunk DMAs so MM1 kc=0 starts sooner
                    for kc in range(NKC):
                        eng = nc.sync if kc % 2 == 0 else nc.scalar
                        eng.dma_start(out=xt[:, kc, :], in_=xT[blk, :, kc, :])
                    first_block = False
                else:
                    # half per HWDGE ring: block completes at full fabric BW,
                    # so blocks become ready in consumption order
                    nc.sync.dma_start(
                        out=xt[:, 0:4, :], in_=xT[blk, :, 0:4, :]
                    )
                    nc.scalar.dma_start(
                        out=xt[:, 4:NKC, :], in_=xT[blk, :, 4:NKC, :]
                    )

                # MM1 + fused bias/relu -> h chunks [128, 512]
                hs = []
                for mc in range(2):
                    h_ps = hps.tile([128, NBLK_COLS], f32)
                    for kc in range(NKC):
                        nc.tensor.matmul(
                            h_ps[:],
                            w1_sb[:, kc, mc * 128 : (mc + 1) * 128],
                            xt[:, kc, :],
                            start=(kc == 0),
                            stop=(kc == NKC - 1),
                        )
                    h_sb = hpool.tile([128, NBLK_COLS], H_DT, tag="h")
                    nc.scalar.activation(
                        h_sb[:], h_ps[:], AF.Relu, bias=b1_sb[:, mc : mc + 1]
                    )
                    hs.append(h_sb)
                    if mc == 0 and pending is not None:
                        emit_mm2(*pending)
                        pending = None
                pending = (hs, b0)

            emit_mm2(*pending)

    nc.compile()
    return nc


def _fold_conv_into_w1(conv_w, W1):
    """W1eff[784, 256] such that x @ W1eff == conv(x) flattened @ W1."""
    conv_w = np.asarray(conv_w, dtype=np.float64)
    W1 = np.asarray(W1, dtype=np.float64)
    C = np.zeros((IMG, IMG, OUT_HW, OUT_HW), dtype=np.float64)
    oi = np.arange(OUT_HW)[:, None]
    oj = np.arange(OUT_HW)[None, :]
    for ki in range(KSZ):
        for kj in range(KSZ):
            C[oi + ki, oj + kj, oi, oj] = conv_w[ki, kj]
    W1eff = C.reshape(PIX, FEAT) @ W1
    return np.ascontiguousarray(W1eff, dtype=np.float32)


def _pack_weights(w1e, b1, W2, b2):
    # w1 blob [112, 7, 256]: chunk-major per partition (matches w1_sb)
    np_wdt = mybir.dt.np(W_DT)
    w1b = np.ascontiguousarray(
        w1e.reshape(NKC, KCH, HID).transpose(1, 0, 2).astype(np_wdt)
    )
    w2b = np.ascontiguousarray(
        W2.reshape(2, 128, NCLS).transpose(1, 0, 2).astype(np_wdt)
    )
    wb = np.zeros((128, WB_W), dtype=np.float32)
    wb[:, WB_B1:WB_B2] = b1.reshape(2, 128).T
    wb[:NCLS, WB_B2] = b2
    return w1b, w2b, wb


def kernel(x, conv_w, W1, b1, W2, b2, _bc=BC, _trace=False):
    x = np.asarray(x, dtype=np.float32)
    w1e = _fold_conv_into_w1(conv_w, W1)
    b1 = np.asarray(b1, dtype=np.float32)
    W2 = np.asarray(W2, dtype=np.float32)
    b2 = np.asarray(b2, dtype=np.float32)
    w1b, w2b, wb = _pack_weights(w1e, b1, W2, b2)

    n_cores = x.shape[0] // _bc
    if _bc not in _CACHE:
        _CACHE[_bc] = _build(_bc)
    nc = _CACHE[_bc]

    nblk = _bc // NBLK_COLS
    in_maps = [
        {
            # [nblk, 512, 7, 112] -> [nblk, 112, 7, 512]; see _build
            "xT": np.ascontiguousarray(
                x[c * _bc : (c + 1) * _bc]
                .reshape(nblk, NBLK_COLS, NKC, KCH)
                .transpose(0, 3, 2, 1)
                .astype(mybir.dt.np(X_DT))
            ),
            "w1b": w1b,
            "w2b": w2b,
            "wb": wb,
        }
        for c in range(n_cores)
    ]
    res = run_bass_kernel_spmd(
        nc, in_maps, core_ids=list(range(n_cores)), trace=_trace
    )
    # device layout logitsT [10, bc] -> [bc, 10]
    out = np.concatenate(
        [np.ascontiguousarray(res.results[c]["out"].T) for c in range(n_cores)],
        axis=0,
    )
    if _trace:
        return out, res
    return out
